# revision 13
# baseline (speedup 1.0000x reference)
"""CrossAttentionHead TRN2 kernel (v4).

Full inputs -> full output. Shards batch (B=8) across 8 NeuronCores,
one batch element per core (pure data parallel, no collectives).

Per-core layout (xT staged host-side as bf16 [E=768, S=2048]):
  qT/kT/vT = W*.T @ xT + b*          ([H=128, S], weights stationary, bf16)
  vN       = blockwise transpose(vT)  ([S,H] natural, bf16)
  scores   sT[sk, sq] = kT_blk.T @ qT (psum f32, 1024-sq halves, ring of 2)
  es       = exp(sT * 1/sqrt(768))    (ScalarE, bf16 out)
  acc     += es                       (DVE bf16, for row sums)
  oT      += vN_blk.T @ es            (PV accumulate, [H, S] psum f32)
  rowsumT  = acc_blk.T @ ones         (PE, [sq,1] per 128-block -> [128,16])
  out      = transpose(oT) * (1/rowsumT)  (normalize fused into ACT scale)

Scheduling: q&k projections run chunk-outer so the PE chases the xT DMA
stream with no idle; v runs after (q's psum banks reused); all drains are
split across ScalarE/DVE slices off the critical path. Identity matrices
come in via DMA (GpSimd-generated constants gate the PE by ~7 us
otherwise). Bulk DMAs issue on the GpSimd queue: its DGE issue cost is
~25 ns vs ~1.2 us on the Sync queue, which otherwise serializes both the
input stream and the 16 output-block stores.

Matmuls stream 1 cyc/col at 2.4 GHz regardless of bf16/f32r; bf16 is for
halved DMA/SBUF traffic and 2x DVE. The main loop is ScalarE-bound: exp
of 4.19M elements ~ 32 us, pipelined against PE scores+AV (27 us).
Softmax skips max-subtraction: energy/sqrt(768) ~ N(0, 0.41^2).
Measured numerics vs fp32 reference: rel err ~4e-3 (gate 2e-2).
"""

import sys

if '/opt/trn_rl_repo' not in sys.path:
    sys.path.insert(0, '/opt/trn_rl_repo')

import numpy as np

B, S, E, H = 8, 2048, 768, 128
NCORES = 8
ST = S // 128           # 16 sk tiles
EC = E // 128           # 6 embed chunks
SCALE = float(1.0 / np.sqrt(np.float32(E)))

_CACHE = {}


def _build():
    import concourse.bacc as bacc
    import concourse.mybir as mybir
    import concourse.tile as tile

    dt = mybir.dt
    f32 = dt.float32
    bf16 = dt.bfloat16
    AF = mybir.ActivationFunctionType

    nc = bacc.Bacc(None, target_bir_lowering=False)
    xT_d = nc.dram_tensor("xT", [E, S], dt.uint16, kind="ExternalInput")
    idf_d = nc.dram_tensor("identf", [128, 128], f32, kind="ExternalInput")
    idb_d = nc.dram_tensor("identb", [128, 128], dt.uint16,
                           kind="ExternalInput")
    w_d = {}
    b_d = {}
    for nm in ("q", "k", "v"):
        w_d[nm] = nc.dram_tensor(f"W{nm}", [E, H], dt.uint16,
                                 kind="ExternalInput")
        b_d[nm] = nc.dram_tensor(f"b{nm}", [H], f32, kind="ExternalInput")
    out_d = nc.dram_tensor("out", [S, H], f32, kind="ExternalOutput")

    with tile.TileContext(nc) as tc:
        with tc.tile_pool(name="data", bufs=1) as db, \
             tc.tile_pool(name="es", bufs=3) as esp:
            identf = db.tile([128, 128], f32, name="identf")
            identb = db.tile([128, 128], bf16, name="identb")
            onesb = db.tile([128, 1], bf16, name="onesb")
            nc.vector.memset(onesb[:], 1.0)

            # DMA issue plan ordered by need-time: chunk0 + q/k weights
            # first on sync; Wv and late chunks on scalar; constants last.
            xT = [db.tile([128, S], bf16, name=f"xT{c}") for c in range(EC)]
            w_sb = {}
            b_sb = {}
            for nm in ("q", "k", "v"):
                w_sb[nm] = db.tile([128, EC, H], bf16, name=f"w_{nm}")

            def w_dma(eng, nm):
                eng.dma_start(
                    out=w_sb[nm][:],
                    in_=w_d[nm].rearrange("(c p) d -> p c d", p=128)
                    .bitcast(bf16))

            def x_dma(eng, c):
                eng.dma_start(
                    out=xT[c][:],
                    in_=xT_d[c * 128:(c + 1) * 128, :].bitcast(bf16))

            x_dma(nc.sync, 0)
            w_dma(nc.scalar, "q")
            w_dma(nc.scalar, "k")
            x_dma(nc.sync, 1)
            x_dma(nc.scalar, 2)
            x_dma(nc.sync, 3)
            x_dma(nc.scalar, 4)
            x_dma(nc.sync, 5)
            w_dma(nc.scalar, "v")
            for nm in ("q", "k", "v"):
                b_sb[nm] = db.tile([128, 1], f32, name=f"b_{nm}")
                nc.sync.dma_start(out=b_sb[nm][:], in_=b_d[nm][:, None])
            nc.gpsimd.dma_start(out=identb[:], in_=idb_d[:, :].bitcast(bf16))
            nc.gpsimd.dma_start(out=identf[:], in_=idf_d[:, :])

            qT = db.tile([128, S], bf16, name="qT")
            kT = db.tile([128, S], bf16, name="kT")
            vT = db.tile([128, S], bf16, name="vT")
            vN = db.tile([128, S], bf16, name="vN")
            acc = db.tile([128, S], bf16, name="acc")
            oT_sb = db.tile([128, S], bf16, name="oT_sb")
            rcpT = db.tile([128, ST], f32, name="rcpT")

            # ---- warm-up: ramp the PE clock while DMAs stream.
            # Junk operands from a DVE memset so no DMA gates the PE. ----
            wj = db.tile([128, 512], bf16, name="wjunk")
            nc.vector.memset(wj[:], 0.03125)
            with tc.tile_pool(name="pw", bufs=1, space="PSUM",
                              side="left") as pw:
                wps = pw.tile([128, 512], f32, tag="w")
                for _ in range(9):
                    nc.tensor.matmul(wps[:], wj[:, :128], wj[:],
                                     start=True, stop=True)
                wsink = db.tile([128, 512], f32, name="wsink")
                nc.vector.tensor_copy(wsink[:], wps[:])

            # ---- projections: q,k chunk-outer chasing the DMA stream ----
            pq_cm = tc.tile_pool(name="pq", bufs=1, space="PSUM", side="left")
            pq = pq_cm.__enter__()
            q_ps = pq.tile([128, S], f32, tag="q")
            pk_cm = tc.tile_pool(name="pk", bufs=1, space="PSUM", side="right")
            pk = pk_cm.__enter__()
            k_ps = pk.tile([128, S], f32, tag="k")
            for c in range(EC):
                for nm, ps in (("q", q_ps), ("k", k_ps)):
                    for n in range(4):
                        nc.tensor.matmul(
                            ps[:, n * 512:(n + 1) * 512],
                            w_sb[nm][:, c, :],
                            xT[c][:, n * 512:(n + 1) * 512],
                            start=(c == 0), stop=(c == EC - 1))
            # drains off the PE critical path: q then k, scalar+DVE halves
            nc.scalar.activation(qT[:, :1024], q_ps[:, :1024], AF.Identity,
                                 bias=b_sb["q"][:], scale=1.0)
            nc.vector.tensor_scalar_add(qT[:, 1024:], q_ps[:, 1024:],
                                        b_sb["q"][:])
            pq_cm.__exit__(None, None, None)

            pv_cm = tc.tile_pool(name="pv", bufs=1, space="PSUM", side="left")
            pv = pv_cm.__enter__()
            v_ps = pv.tile([128, S], f32, tag="v")
            for c in range(EC):
                for n in range(4):
                    nc.tensor.matmul(
                        v_ps[:, n * 512:(n + 1) * 512],
                        w_sb["v"][:, c, :],
                        xT[c][:, n * 512:(n + 1) * 512],
                        start=(c == 0), stop=(c == EC - 1))
            nc.scalar.activation(kT[:, :1024], k_ps[:, :1024], AF.Identity,
                                 bias=b_sb["k"][:], scale=1.0)
            nc.vector.tensor_scalar_add(kT[:, 1024:], k_ps[:, 1024:],
                                        b_sb["k"][:])
            pk_cm.__exit__(None, None, None)

            # v drain in 4 slices alternating scalar/DVE
            for n in range(4):
                sl = slice(n * 512, (n + 1) * 512)
                if n % 2 == 0:
                    nc.scalar.activation(vT[:, sl], v_ps[:, sl], AF.Identity,
                                         bias=b_sb["v"][:], scale=1.0)
                else:
                    nc.vector.tensor_scalar_add(vT[:, sl], v_ps[:, sl],
                                                b_sb["v"][:])
            pv_cm.__exit__(None, None, None)

            # ---- main attention ----
            ps_cm = tc.tile_pool(name="ps", bufs=2, space="PSUM", side="left")
            psl = ps_cm.__enter__()

            def emit_scores(kt, h):
                t = psl.tile([128, 1024], f32, tag="s")
                for n in range(2):
                    q0 = h * 1024 + n * 512
                    nc.tensor.matmul(
                        t[:, n * 512:(n + 1) * 512],
                        kT[:, kt * 128:(kt + 1) * 128],
                        qT[:, q0:q0 + 512],
                        start=True, stop=True)
                return t

            s_half = [emit_scores(0, 0), emit_scores(0, 1)]

            # vN transposes (PE) while drains finish; ring on the right
            with tc.tile_pool(name="pvnt", bufs=2, space="PSUM",
                              side="right") as pvnt:
                for j in range(ST):
                    pt = pvnt.tile([128, 128], bf16, tag="vt")
                    nc.tensor.transpose(
                        pt[:], vT[:, j * 128:(j + 1) * 128], identb[:])
                    nc.vector.tensor_copy(vN[:, j * 128:(j + 1) * 128], pt[:])

            poT_cm = tc.tile_pool(name="poT", bufs=1, space="PSUM",
                                  side="right")
            poT = poT_cm.__enter__()
            oT_ps = poT.tile([128, S], f32, tag="o")

            es_last = {}
            for kt in range(ST):
                es = esp.tile([128, S], bf16, tag="es")
                es_last[kt] = es
                for h in range(2):
                    nc.scalar.activation(
                        es[:, h * 1024:(h + 1) * 1024], s_half[h][:],
                        AF.Exp, scale=SCALE)
                # the last two tiles skip the DVE accumulate: their column
                # sums fold into the rowsumT matmul groups instead, so the
                # tail is not serialized behind a DVE add after exp15
                if kt == 0:
                    nc.vector.tensor_copy(acc[:], es[:])
                elif kt < ST - 2:
                    nc.vector.tensor_add(acc[:], acc[:], es[:])
                if kt < ST - 1:
                    s_half = [emit_scores(kt + 1, 0), emit_scores(kt + 1, 1)]
                for n in range(4):
                    nc.tensor.matmul(
                        oT_ps[:, n * 512:(n + 1) * 512],
                        vN[:, kt * 128:(kt + 1) * 128],
                        es[:, n * 512:(n + 1) * 512],
                        start=(kt == 0), stop=(kt == ST - 1))
            ps_cm.__exit__(None, None, None)

            # ---- finale ----
            pf_cm = tc.tile_pool(name="pf", bufs=1, space="PSUM", side="left")
            pf = pf_cm.__enter__()
            rsT_ps = pf.tile([128, ST], f32, tag="rs")
            for j in range(ST):
                jb = slice(j * 128, (j + 1) * 128)
                nc.tensor.matmul(rsT_ps[:, j:j + 1], acc[:, jb],
                                 onesb[:], start=True, stop=False)
                nc.tensor.matmul(rsT_ps[:, j:j + 1], es_last[ST - 2][:, jb],
                                 onesb[:], start=False, stop=False)
                nc.tensor.matmul(rsT_ps[:, j:j + 1], es_last[ST - 1][:, jb],
                                 onesb[:], start=False, stop=True)
            nc.vector.reciprocal(rcpT[:], rsT_ps[:])

            # oT psum -> SBUF bf16 (4 slices alternating scalar/DVE)
            for n in range(4):
                sl = slice(n * 512, (n + 1) * 512)
                if n % 2 == 0:
                    nc.scalar.activation(oT_sb[:, sl], oT_ps[:, sl],
                                         AF.Identity, scale=1.0)
                else:
                    nc.vector.tensor_copy(oT_sb[:, sl], oT_ps[:, sl])
            poT_cm.__exit__(None, None, None)

            # transpose blocks in groups of 4, one wide broadcast-multiply
            # per group, one batched store DMA per group
            stage = db.tile([128, S], f32, name="stage")
            with tc.tile_pool(name="pft", bufs=3, space="PSUM",
                              side="left") as pft:
                for g in range(8):
                    ftw = pft.tile([128, 256], bf16, tag="ftw")
                    for i in range(2):
                        st = g * 2 + i
                        nc.tensor.transpose(
                            ftw[:, i * 128:(i + 1) * 128],
                            oT_sb[:, st * 128:(st + 1) * 128], identb[:])
                    gsl = slice(g * 256, (g + 1) * 256)
                    nc.vector.tensor_mul(
                        stage[:, gsl].rearrange("p (t h) -> p t h", t=2),
                        ftw[:].rearrange("p (t h) -> p t h", t=2),
                        rcpT[:, g * 2:(g + 1) * 2, None]
                        .broadcast_to([128, 2, H]))
                    eng = nc.sync if g % 2 == 0 else nc.scalar
                    eng.dma_start(
                        out=out_d[g * 256:(g + 1) * 256, :]
                        .rearrange("(t p) d -> p t d", p=128),
                        in_=stage[:, gsl].rearrange("p (t h) -> p t h", t=2))
            pf_cm.__exit__(None, None, None)

    nc.finalize()
    return nc


def _get_nc():
    if "nc" not in _CACHE:
        _CACHE["nc"] = _build()
    return _CACHE["nc"]


def make_in_maps(x, Wq, bq, Wk, bk, Wv, bv):
    import ml_dtypes

    bf = ml_dtypes.bfloat16
    x = np.asarray(x, dtype=np.float32)
    eye = np.eye(128, dtype=np.float32)
    shared = {
        "identf": eye,
        "identb": eye.astype(bf).view(np.uint16),
        "Wq": np.asarray(Wq, np.float32).astype(bf).view(np.uint16),
        "bq": np.asarray(bq, np.float32),
        "Wk": np.asarray(Wk, np.float32).astype(bf).view(np.uint16),
        "bk": np.asarray(bk, np.float32),
        "Wv": np.asarray(Wv, np.float32).astype(bf).view(np.uint16),
        "bv": np.asarray(bv, np.float32),
    }
    in_maps = []
    for b in range(NCORES):
        xTb = np.ascontiguousarray(x[b].T).astype(bf).view(np.uint16)
        in_maps.append({"xT": xTb, **shared})
    return in_maps


def kernel(x, enc_output, Wq, bq, Wk, bk, Wv, bv):
    from concourse.bass_utils import run_bass_kernel_spmd

    nc = _get_nc()
    in_maps = make_in_maps(x, Wq, bq, Wk, bk, Wv, bv)
    res = run_bass_kernel_spmd(nc, in_maps, list(range(NCORES)))
    out = np.stack([res.results[b]["out"] for b in range(NCORES)], axis=0)
    return out.astype(np.float32)
